# revision 33
# baseline (speedup 1.0000x reference)
"""Trainium2 Bass kernel for multi-head attention (GQA + RoPE + causal).

Problem shapes (hardcoded): x [2, 2048, 2048] f32, w_qkv [3072, 2048],
w_o [2048, 2048], position_ids [2, 2048] int, mask [1,1,2048,2048] causal.

Sharding: 8 cores = 2 batches x 4 KV-head groups. Each core computes, for
one batch b and one kv-group g (4 query heads + 1 kv head):
  - Y^T = (w_shard @ x[b]^T) in "feature-major" layout [f, s] (bf16 matmuls)
  - RoPE on Q^T/K^T (tables precomputed on host from position_ids)
  - causal attention in transposed-score layout S_T[k, q] (no transposes,
    softmax sums via ones-vector matmul; no max subtraction needed at these
    score magnitudes |s| < ~10)
  - partial o_proj out^T[oc, s] = w_o_slice^T @ A^T  (f32 partial)
Host sums the 4 partials per batch and transposes back.
"""

import math
from contextlib import ExitStack
from dataclasses import dataclass

import numpy as np
import ml_dtypes

import concourse.bass as bass
import concourse.tile as tile
from concourse import bacc, mybir
from concourse.masks import make_identity

P = 128
BF16 = mybir.dt.bfloat16
F32 = mybir.dt.float32
BF16_NP = ml_dtypes.bfloat16

# full-size problem constants
B, S_FULL, HID_FULL = 2, 2048, 2048
NH, NKV, HD = 16, 4, 128
NQL_HD = (NH // NKV) * HD  # 512
ROPE_BASE = 10000.0
N_CORES = 8


@dataclass(frozen=True)
class Cfg:
    S: int = S_FULL          # sequence length
    HID: int = HID_FULL      # model dim (contraction for qkv proj)
    NQL: int = NH // NKV     # local query heads per core
    QT: int = 512            # q tile (matmul free dim)
    KG: int = 1              # k-tiles per exp group

    @property
    def HT(self):            # contraction tiles for qkv proj
        return self.HID // P

    @property
    def NQT(self):           # q tiles per head
        return self.S // self.QT

    @property
    def NKT(self):           # k tiles (128 wide)
        return self.S // P

    @property
    def FQK(self):           # 128-blocks of qk features (NQL q heads + 1 k head)
        return self.NQL + 1

    @property
    def OC(self):            # o_proj output features (full hidden)
        return self.HID

    @property
    def TPQ(self):           # k tiles per q tile (causal step)
        return self.QT // P


def emit(ctx: ExitStack, tc: tile.TileContext, cfg: Cfg, io: dict, n_reps: int = 1):
    res = ctx.enter_context(tc.tile_pool(name="res", bufs=1))
    work = ctx.enter_context(tc.tile_pool(name="work", bufs=1))
    ps = ctx.enter_context(tc.tile_pool(name="ps", bufs=1, space="PSUM"))
    for rep in range(n_reps):  # >1 only for timing builds
        # accumulate into outT on reps > 0 so repeats aren't dead-code
        # eliminated by the NEFF compiler (timing builds only)
        emit_once(tc, cfg, io, res, work, ps, accum=(rep > 0))


def emit_once(tc: tile.TileContext, cfg: Cfg, io: dict, res, work, ps, accum=False):
    nc = tc.nc
    S, QT, KG, HT, NQL = cfg.S, cfg.QT, cfg.KG, cfg.HT, cfg.NQL
    NS = S // QT  # s slices of size QT for projection loops
    xT, wqkT, wvT, woT, cosT, sinT, ones_col, outT = (
        io["xT"], io["wqkT"], io["wvT"], io["woT"], io["cosT"], io["sinT"],
        io["ones_col"], io["outT"],
    )
    tri = io["tri"]

    # chunked loads (per h-tile) so the first matmuls can start early
    xT_sb = res.tile([P, HT, S], BF16, tag="xt")
    xT_r = xT.rearrange("(ht p) s -> p ht s", p=P)
    wqk_sb = res.tile([P, HT, cfg.FQK * P], BF16, tag="wqk")
    wqk_r = wqkT.rearrange("(ht p) f -> p ht f", p=P)
    wv_sb = res.tile([P, HT, P], BF16, tag="wv")
    wv_r = wvT.rearrange("(ht p) f -> p ht f", p=P)
    # DMA order matters: the projections can only finish once the LAST xT
    # chunk lands, so keep everything except the small K-slice of wqk off
    # the queue until xT is fully issued.
    CH = max(1, HT // 8)  # 8 chunks
    kf = bass.ts(NQL, P)  # K feature columns of wqk
    for h0 in range(0, HT, CH):
        hs = slice(h0, h0 + CH)
        nc.sync.dma_start(out=wqk_sb[:, hs, kf], in_=wqk_r[:, hs, kf])
        nc.sync.dma_start(out=xT_sb[:, hs, :], in_=xT_r[:, hs, :])
    qf = slice(0, NQL * P)  # Q feature columns
    for h0 in range(0, HT, CH):
        hs = slice(h0, h0 + CH)
        nc.sync.dma_start(out=wv_sb[:, hs, :], in_=wv_r[:, hs, :])
        nc.sync.dma_start(out=wqk_sb[:, hs, qf], in_=wqk_r[:, hs, qf])
    cos_sb = res.tile([P, S], BF16, tag="cos")
    sin_sb = res.tile([P, S], BF16, tag="sin")
    nc.sync.dma_start(out=cos_sb[:], in_=cosT[:, :])
    nc.sync.dma_start(out=sin_sb[:], in_=sinT[:, :])
    ones_c_sb = res.tile([P, 1], BF16, tag="onesc")
    nc.sync.dma_start(out=ones_c_sb[:], in_=ones_col[:, :])
    tri_sb = res.tile([P, cfg.TPQ, QT], BF16, tag="tri")
    nc.sync.dma_start(out=tri_sb[:], in_=tri.rearrange("(d p) q -> p d q", p=P))
    ident_sb = res.tile([P, P], BF16, tag="ident")
    make_identity(nc, ident_sb[:])
    wo_sb = res.tile([P, NQL, cfg.OC], BF16, tag="wo")
    nc.sync.dma_start(out=wo_sb[:], in_=woT.rearrange("(fq p) oc -> p fq oc", p=P))

    qT_sb = res.tile([P, NQL, S], BF16, tag="qT")   # roped, pre-scaled Q^T
    kT_sb = res.tile([P, S], BF16, tag="kT")        # roped K^T
    v_sb = res.tile([P, cfg.NKT, P], BF16, tag="v")  # V natural [s-part, v]
    a_sb = res.tile([P, NQL, S], BF16, tag="a")     # attention out A^T

    # ---- projection helper (Y^T for one 128-wide feature block) ----
    def proj_block(w_sb, fslice, si, dst, do_rope):
        sl = bass.ts(si, QT)
        acc = ps.tile([P, QT], F32, tag="mm", bufs=2, name="acc")
        for hi in range(HT):
            nc.tensor.matmul(
                acc[:], w_sb[:, hi, fslice], xT_sb[:, hi, sl],
                start=(hi == 0), stop=(hi == HT - 1),
            )
        y = work.tile([P, QT], BF16, tag="y", bufs=4, name="y")
        nc.scalar.copy(y[:], acc[:])
        if not do_rope:
            return y
        # rope: out = y*cos + swap_halves(y)*sin'
        # (sin' is pre-negated in its lower half on host).
        # Half-swap via 1-input copies: 2-input DVE ops require equal
        # SBUF base partitions on HW.
        sw = work.tile([P, QT], BF16, tag="sw", bufs=3, name="sw")
        nc.vector.tensor_copy(sw[0:64, :], y[64:128, :])
        nc.vector.tensor_copy(sw[64:128, :], y[0:64, :])
        t1 = work.tile([P, QT], BF16, tag="t1", bufs=3, name="t1")
        nc.vector.tensor_mul(t1[:], sw[:], sin_sb[:, sl])
        t2 = work.tile([P, QT], BF16, tag="t2", bufs=3, name="t2")
        nc.vector.tensor_mul(t2[:], y[:], cos_sb[:, sl])
        nc.vector.tensor_add(dst, t2[:], t1[:])
        return None

    # ---- K^T projection first (attention needs it before q heads) ----
    with nc.named_scope("k_proj"):
        for si in range(NS):
            proj_block(wqk_sb, bass.ts(NQL, P), si, kT_sb[:, bass.ts(si, QT)], True)

    # ---- V^T projection + transpose to natural V ----
    with nc.named_scope("v_proj"):
        for si in range(NS):
            vt = proj_block(wv_sb, slice(0, P), si, None, False)
            for j in range(QT // P):
                pst = ps.tile([P, P], BF16, tag="mm", bufs=2, name="pst")
                nc.tensor.transpose(pst[:], vt[:, bass.ts(j, P)], ident_sb[:])
                nc.scalar.copy(v_sb[:, si * (QT // P) + j, :], pst[:])

    # ---- Q^T projections ----
    with nc.named_scope("q_proj"):
        for fi in range(NQL):
            for si in range(NS):
                proj_block(wqk_sb, bass.ts(fi, P), si,
                           qT_sb[:, fi, bass.ts(si, QT)], True)

    # ---- attention + o_proj interleaved per q tile ----
    with nc.named_scope("attn"):
        for t in range(cfg.NQT):
            qsl = bass.ts(t, QT)
            for h in range(NQL):
                nk = (t + 1) * cfg.TPQ  # valid k tiles (causal)
                groups = [list(range(g, min(g + KG, nk))) for g in range(0, nk, KG)]
                pv_ps = ps.tile([P, QT], F32, tag="pv", bufs=2, name="pv_ps")
                sums_ps = ps.tile([1, QT], F32, tag="sums", bufs=1, name="sums_ps")
                first = True
                for ks in groups:
                    s_ps = ps.tile([P, KG, QT], F32, tag="s", bufs=3, name="s_ps")
                    p_sb = work.tile([P, KG, QT], BF16, tag="p", bufs=4, name="p_sb")
                    # boundary tiles (delta >= 0): columns q' < delta are fully
                    # masked, so restrict the whole chain to [delta:QT].
                    deltas = [max(0, j * P - t * QT) for j in ks]
                    for i, j in enumerate(ks):
                        d = deltas[i]
                        nc.tensor.matmul(
                            s_ps[:, i, d:QT],
                            kT_sb[:, bass.ts(j, P)],
                            qT_sb[:, h, t * QT + d:(t + 1) * QT],
                            start=True, stop=True,
                        )
                        nc.scalar.activation(
                            p_sb[:, i, d:QT], s_ps[:, i, d:QT],
                            mybir.ActivationFunctionType.Exp,
                        )
                        if j * P - t * QT >= 0:
                            # diagonal 128-block keeps q' >= k' + delta, i.e.
                            # the base (delta=0) triangle at offset delta
                            nc.vector.tensor_mul(
                                p_sb[:, i, d:d + P], p_sb[:, i, d:d + P],
                                tri_sb[:, 0, 0:P],
                            )
                    for i, j in enumerate(ks):
                        d = deltas[i]
                        last = j == nk - 1
                        nc.tensor.matmul(
                            sums_ps[:, d:QT], ones_c_sb[:, :], p_sb[:, i, d:QT],
                            start=first, stop=last,
                        )
                        nc.tensor.matmul(
                            pv_ps[:, d:QT], v_sb[:, j, :], p_sb[:, i, d:QT],
                            start=first, stop=last,
                        )
                        first = False
                recip = work.tile([1, QT], F32, tag="recip", bufs=2, name="recip")
                nc.vector.reciprocal(recip[:], sums_ps[:])
                bc_sb = work.tile([P, QT], F32, tag="bc", bufs=2, name="bc_sb")
                nc.gpsimd.partition_broadcast(bc_sb[:], recip[:], channels=P)
                nc.vector.tensor_mul(a_sb[:, h, qsl], pv_ps[:], bc_sb[:])
            # o_proj for this q tile (all output features)
            for oi in range(cfg.OC // P):
                acc = ps.tile([P, QT], F32, tag="mm", bufs=2, name="acc_o")
                for fi in range(NQL):
                    nc.tensor.matmul(
                        acc[:], wo_sb[:, fi, bass.ts(oi, P)], a_sb[:, fi, qsl],
                        start=(fi == 0), stop=(fi == NQL - 1),
                    )
                orow = work.tile([P, QT], F32, tag="orow", bufs=4, name="orow")
                if accum and oi == 0 and t == 0:
                    # timing builds: chain on previous rep's output so the
                    # NEFF compiler can't dead-code-eliminate earlier reps
                    prev = work.tile([P, QT], F32, tag="prev", bufs=1, name="prev")
                    nc.sync.dma_start(out=prev[:], in_=outT[0:P, 0:QT])
                    nc.vector.tensor_add(orow[:], acc[:], prev[:])
                else:
                    nc.vector.tensor_copy(orow[:], acc[:])
                nc.sync.dma_start(out=outT[bass.ts(oi, P), qsl], in_=orow[:])


def build(cfg: Cfg, n_reps: int = 1):
    nc = bacc.Bacc("TRN2", target_bir_lowering=False, debug=False)
    io = {
        "xT": nc.dram_tensor("xT", [cfg.HID, cfg.S], BF16, kind="ExternalInput").ap(),
        "wqkT": nc.dram_tensor("wqkT", [cfg.HID, cfg.FQK * P], BF16, kind="ExternalInput").ap(),
        "wvT": nc.dram_tensor("wvT", [cfg.HID, P], BF16, kind="ExternalInput").ap(),
        "woT": nc.dram_tensor("woT", [cfg.NQL * P, cfg.OC], BF16, kind="ExternalInput").ap(),
        "cosT": nc.dram_tensor("cosT", [P, cfg.S], BF16, kind="ExternalInput").ap(),
        "sinT": nc.dram_tensor("sinT", [P, cfg.S], BF16, kind="ExternalInput").ap(),
        "ones_col": nc.dram_tensor("ones_col", [P, 1], BF16, kind="ExternalInput").ap(),
        "tri": nc.dram_tensor("tri", [(cfg.QT // P) * P, cfg.QT], BF16, kind="ExternalInput").ap(),
        "outT": nc.dram_tensor("outT", [cfg.OC, cfg.S], F32, kind="ExternalOutput").ap(),
    }
    with tile.TileContext(nc) as tc:
        with ExitStack() as ctx:
            emit(ctx, tc, cfg, io, n_reps=n_reps)
    nc.compile()
    return nc


def rope_tables(position_ids_b: np.ndarray):
    """cos/sin tables in [d, s] layout, both halves stacked; sin lower half
    negated (so rope = y*cos + swap(y)*sin)."""
    half = HD // 2
    inv_freq = 1.0 / (ROPE_BASE ** (np.arange(half, dtype=np.float64) / half))
    freqs = np.asarray(position_ids_b, dtype=np.float64)[None, :] * inv_freq[:, None]
    cos = np.cos(freqs)
    sin = np.sin(freqs)
    cosT = np.concatenate([cos, cos], 0)
    sinT = np.concatenate([-sin, sin], 0)
    return cosT, sinT


def make_in_maps(x, position_ids, w_qkv, w_o):
    """Shard full inputs into per-core input maps (host-side prep)."""
    q_dim = NH * HD
    kv_dim = NKV * HD
    in_maps = []
    ones_col = np.ones((P, 1), dtype=BF16_NP)
    tri = make_tri(512)
    scale = 1.0 / math.sqrt(HD)
    tabs = {}
    for b in range(B):
        cosT, sinT = rope_tables(position_ids[b])
        tabs[b] = (cosT.astype(BF16_NP), sinT.astype(BF16_NP))
    for c in range(N_CORES):
        b, g = divmod(c, NKV)
        # weights for this core's heads: 4 q heads (pre-scaled), 1 k, 1 v head
        wq = w_qkv[g * NQL_HD:(g + 1) * NQL_HD, :] * scale
        wk = w_qkv[q_dim + g * HD:q_dim + (g + 1) * HD, :]
        wv = w_qkv[q_dim + kv_dim + g * HD:q_dim + kv_dim + (g + 1) * HD, :]
        wqkT = np.ascontiguousarray(np.concatenate([wq, wk], 0).T).astype(BF16_NP)
        wvT = np.ascontiguousarray(wv.T).astype(BF16_NP)
        # o_proj: rows of w_o^T for this core's flattened head features
        woT = np.ascontiguousarray(w_o.T[g * NQL_HD:(g + 1) * NQL_HD, :]).astype(BF16_NP)
        in_maps.append({
            "xT": np.ascontiguousarray(x[b].T).astype(BF16_NP),
            "wqkT": wqkT,
            "wvT": wvT,
            "woT": woT,
            "cosT": tabs[b][0],
            "sinT": tabs[b][1],
            "ones_col": ones_col,
            "tri": tri,
        })
    return in_maps


def make_tri(QT):
    """Stacked boundary masks: tri[d*128+k, q] = 1 if q >= k + d*128."""
    k = np.arange(P)
    q = np.arange(QT)
    blocks = [(q[None, :] >= (k[:, None] + d)) for d in range(0, QT, P)]
    return np.concatenate(blocks, 0).astype(BF16_NP)


def _causal_mask_ok(mask):
    m = np.asarray(mask)
    if m.shape != (1, 1, S_FULL, S_FULL):
        return False
    tril = np.tril(np.ones((S_FULL, S_FULL), dtype=bool))
    m0 = m[0, 0]
    return bool((m0[tril] == 0.0).all() and (m0[~tril] <= -1e8).all())


def _reference_numpy(x, position_ids, mask, w_qkv, w_o):
    """Fallback (never expected to trigger): plain numpy reference."""
    half = HD // 2

    def rope(v, pos):
        inv_freq = 1.0 / (ROPE_BASE ** (np.arange(half) / half))
        f = np.asarray(pos, dtype=np.float64)[:, None] * inv_freq[None, :]
        cos, sin = np.cos(f), np.sin(f)
        x1, x2 = v[..., :half], v[..., half:]
        return np.concatenate([x1 * cos - x2 * sin, x2 * cos + x1 * sin], -1)

    out = np.empty((B, S_FULL, HID_FULL), np.float32)
    q_dim, kv_dim = NH * HD, NKV * HD
    xd = x.astype(np.float64)
    for b in range(B):
        qkv = xd[b] @ w_qkv.T.astype(np.float64)
        q = qkv[:, :q_dim].reshape(S_FULL, NH, HD).transpose(1, 0, 2)
        k = qkv[:, q_dim:q_dim + kv_dim].reshape(S_FULL, NKV, HD).transpose(1, 0, 2)
        v = qkv[:, q_dim + kv_dim:].reshape(S_FULL, NKV, HD).transpose(1, 0, 2)
        q = np.stack([rope(qh, position_ids[b]) for qh in q])
        k = np.stack([rope(kh, position_ids[b]) for kh in k])
        rep = NH // NKV
        acc = np.empty((S_FULL, NH, HD))
        for h in range(NH):
            s = q[h] @ k[h // rep].T / math.sqrt(HD) + mask[0, 0]
            s -= s.max(-1, keepdims=True)
            e = np.exp(s)
            p = e / e.sum(-1, keepdims=True)
            acc[:, h, :] = p @ v[h // rep]
        out[b] = (acc.reshape(S_FULL, NH * HD) @ w_o.T.astype(np.float64)).astype(np.float32)
    return out


_NC_CACHE = {}


def _get_nc():
    if "full" not in _NC_CACHE:
        _NC_CACHE["full"] = build(Cfg())
    return _NC_CACHE["full"]


def kernel(x, position_ids, mask, w_qkv, w_o):
    x = np.asarray(x, dtype=np.float32)
    position_ids = np.asarray(position_ids)
    w_qkv = np.asarray(w_qkv, dtype=np.float32)
    w_o = np.asarray(w_o, dtype=np.float32)
    if not _causal_mask_ok(mask):
        return _reference_numpy(x, position_ids, np.asarray(mask, np.float32),
                                w_qkv, w_o)

    from concourse.bass_utils import run_bass_kernel_spmd

    nc = _get_nc()
    in_maps = make_in_maps(x, position_ids, w_qkv, w_o)
    res = run_bass_kernel_spmd(nc, in_maps, list(range(N_CORES)))
    out = np.empty((B, S_FULL, HID_FULL), dtype=np.float32)
    for b in range(B):
        acc = res.results[b * NKV + 0]["outT"].astype(np.float32)
        for g in range(1, NKV):
            acc = acc + res.results[b * NKV + g]["outT"]
        out[b] = acc.T
    return out
